# revision 14
# baseline (speedup 1.0000x reference)
"""Trainium2 Bass kernel for DistanceGatedScoringFunction (fp8 DoubleRow).

Computation (per row n of the batch):
  gl     = gate_input @ Wg + bg                       [L]
  logits = -(||gl||^2 - 2 gl @ centers.T + ||c||^2)   [E]
  logits = relu(logits @ Wgm1 + bgm1) @ Wgm2 + bgm2   [E]
  probs  = softmax(logits + gumbel)                   [E]
  eo_e   = (relu(relu(x @ We1_e + be1_e) @ We2_e + be2_e)) @ We3_e + be3_e
  out    = sigmoid(sum_e eo_e * probs_e)              [1]

Strategy vs the fp32r baseline (754 us):
 * The dominant K=256 contractions (h1, h2, we3, gate proj) run as fp8
   e4m3 DoubleRow matmuls: one [128,2,M] stationary + [128,2,F] moving
   MM does the whole K=256 contraction at 2 fp8 MACs/cell/cycle.
   All quantization scales are powers of two picked so every PSUM drain
   is a single bias-add(+relu) op: x*32, W1*1 -> psum=32*h1pre,
   h1q=32*relu(h1) (max 104 < 240), W2*2 -> psum=64*h2pre,
   h2q=64*relu(h2) (max 104), W3*32 -> peo=2048*eo_pre.
   Numpy-simulated end-to-end rel err: 1.18e-2 (gate 2e-2).
 * The distance stage is folded into the gating MLP1: pre1 =
   [gl; glsq] @ [2*c^T Wgm1 ; -1 (x) colsum(Wgm1)] + k', so the whole
   gating head is gateproj -> mlp1 -> mlp2.
 * be3 is folded into an extra "nb" column of the den matmul
   (nb = sum_e be3_e w_e); the output is sigmoid((num+nb)/den),
   computed in a [100,128]-batched post-pass.
 * PSUM drains (the relu/bias/downcast ops) are emitted pairwise over
   two 512-token tiles ([128,1024] per instruction) and split between
   the Scalar and Vector engines; den/nb/num rows go PSUM->DRAM by DMA.
 * mlp2 / we3 / den outputs for the two tiles of a pair live at
   partition bases {0,32} (PE column tiling) so the z-add / exp / ewp
   ops each cover both tiles in one instruction.

Data-parallel over 8 NeuronCores (shard N, replicate params); each
core handles 12500 rows padded to 12800 = 25 tiles of 512.
"""

import numpy as np

N, D, H, E, L = 100000, 256, 256, 8, 64
M_CORES = 8
NC_RAW = N // M_CORES       # real rows per core
NC_N = 12800                # padded rows per core (25 tiles of 512)
F = 512                     # token tile (DoubleRow moving free dim limit)


def _build_nc(nc_n, f):
    """Build and compile the single-core Bass program (shared by all cores)."""
    from contextlib import ExitStack

    import concourse.bacc as bacc
    import concourse.mybir as mybir
    import concourse.tile as tile

    fp32 = mybir.dt.float32
    fr = mybir.dt.float32r
    f8 = mybir.dt.float8e4
    AF = mybir.ActivationFunctionType
    OP = mybir.AluOpType
    DR = mybir.MatmulPerfMode.DoubleRow

    t_tiles = nc_n // f
    assert t_tiles * f == nc_n
    n_pairs = t_tiles // 2
    tail = t_tiles % 2
    PP = 128
    PJ = nc_n // PP
    assert PP * PJ == nc_n

    nc = bacc.Bacc("TRN2", target_bir_lowering=False, debug=False)

    # ---- DRAM I/O ----
    xs_d = nc.dram_tensor("xs", [128, 2 * nc_n], f8, kind="ExternalInput")
    xg_d = nc.dram_tensor("xg", [128, 2 * nc_n], f8, kind="ExternalInput")
    gmp_d = nc.dram_tensor("gmp", [8, nc_n], fp32, kind="ExternalInput")
    we1_d = nc.dram_tensor("we1", [128, 16 * 2 * 128], f8, kind="ExternalInput")
    we2_d = nc.dram_tensor("we2", [128, 16 * 2 * 128], f8, kind="ExternalInput")
    we3_d = nc.dram_tensor("we3", [128, 8 * 2 * 16], f8, kind="ExternalInput")
    wgq_d = nc.dram_tensor("wgq", [128, 2 * 128], f8, kind="ExternalInput")
    m1_d = nc.dram_tensor("m1", [128, 256], fr, kind="ExternalInput")
    wgm2_d = nc.dram_tensor("wgm2", [128, 16], fr, kind="ExternalInput")
    bt_d = nc.dram_tensor("bt", [128, 36], fp32, kind="ExternalInput")
    dnw_d = nc.dram_tensor("dnw", [8, 3], fr, kind="ExternalInput")
    out_d = nc.dram_tensor("out", [nc_n], fp32, kind="ExternalOutput")
    scr_d = nc.dram_tensor("scr", [3, nc_n], fp32)  # den/nb/num bounce

    xs_r = xs_d.ap().rearrange("p (i n) -> p i n", i=2)
    xg_r = xg_d.ap().rearrange("p (i n) -> p i n", i=2)
    we1_r = we1_d.ap().rearrange("p (b i m) -> p b i m", b=16, i=2)
    we2_r = we2_d.ap().rearrange("p (b i m) -> p b i m", b=16, i=2)
    we3_r = we3_d.ap().rearrange("p (e i c) -> p e i c", e=8, i=2)
    wgq_r = wgq_d.ap().rearrange("p (i l) -> p i l", i=2)

    with tile.TileContext(nc) as tc, ExitStack() as ctx:
        cw = ctx.enter_context(tc.tile_pool(name="cw", bufs=1))
        xin = ctx.enter_context(tc.tile_pool(name="xin", bufs=3))
        hq = ctx.enter_context(tc.tile_pool(name="hq", bufs=3))
        h2q = ctx.enter_context(tc.tile_pool(name="h2q", bufs=9))
        hgp = ctx.enter_context(tc.tile_pool(name="hgp", bufs=2))
        glp = ctx.enter_context(tc.tile_pool(name="glp", bufs=2))
        wk = ctx.enter_context(tc.tile_pool(name="wk", bufs=3))
        psh = ctx.enter_context(tc.tile_pool(name="psh", bufs=3, space="PSUM"))
        pz = ctx.enter_context(tc.tile_pool(name="pz", bufs=1, space="PSUM"))

        # ---- constants into SBUF (one DMA each) ----
        we1_s = cw.tile([128, 16, 2, 128], f8)
        nc.sync.dma_start(out=we1_s, in_=we1_r)
        we2_s = cw.tile([128, 16, 2, 128], f8)
        nc.sync.dma_start(out=we2_s, in_=we2_r)
        we3_s = cw.tile([128, 8, 2, 16], f8)
        nc.sync.dma_start(out=we3_s, in_=we3_r)
        wgq_s = cw.tile([128, 2, 128], f8)
        nc.sync.dma_start(out=wgq_s, in_=wgq_r)
        m1_s = cw.tile([128, 256], fr)
        nc.sync.dma_start(out=m1_s, in_=m1_d.ap())
        wgm2_s = cw.tile([128, 16], fr)
        nc.sync.dma_start(out=wgm2_s, in_=wgm2_d.ap())
        bt_s = cw.tile([128, 36], fp32)
        nc.sync.dma_start(out=bt_s, in_=bt_d.ap())
        dnw_s = cw.tile([8, 3], fr)
        nc.sync.dma_start(out=dnw_s, in_=dnw_d.ap())

        # round-robin engine picker for the big PSUM relu drains
        # (ScalarE pair-drain ~1.0us vs VectorE ~1.19us -> 9:8 split)
        state = {"i": 0}

        def drain(out_ap, in_ap, bias_ap):
            k = state["i"]
            state["i"] += 1
            if k % 17 in (0, 2, 4, 6, 8, 10, 12, 14, 16):
                nc.scalar.activation(out_ap, in_ap, AF.Relu, bias=bias_ap)
            else:
                nc.vector.tensor_scalar(out=out_ap, in0=in_ap, scalar1=bias_ap,
                                        scalar2=0.0, op0=OP.add, op1=OP.max)

        def do_pair(pidx, nt):
            """Process tiles [2*pidx, 2*pidx+nt) (nt in {1,2})."""
            n0 = pidx * 2 * f
            W = nt * f
            # ---- input tiles ----
            xs_t = xin.tile([128, 2, W], f8, tag="xs", name="xs_t")
            nc.sync.dma_start(out=xs_t, in_=xs_r[:, :, n0 : n0 + W])
            xg_t = xin.tile([128, 2, W], f8, tag="xg", name="xg_t")
            nc.sync.dma_start(out=xg_t, in_=xg_r[:, :, n0 : n0 + W])
            gm_t = xin.tile([8, W], fp32, tag="gm", name="gm_t")
            nc.sync.dma_start(out=gm_t, in_=gmp_d.ap()[:, n0 : n0 + W])

            gs = {}

            def g_gate():
                pgl = psh.tile([128, W], fp32, tag="psh", name="pgl")
                for t in range(nt):
                    sl = slice(t * f, (t + 1) * f)
                    nc.tensor.matmul(pgl[:, sl], wgq_s, xg_t[:, :, sl],
                                     start=True, stop=True, perf_mode=DR)
                glx = glp.tile([128, W], fr, tag="glx", name="glx")
                nc.scalar.activation(glx[0:64, :], pgl[0:64, :], AF.Identity,
                                     bias=bt_s[0:64, 34:35], scale=1.0 / 128.0)
                nc.scalar.activation(glx[64:128, :], pgl[64:128, :], AF.Square,
                                     bias=bt_s[64:128, 34:35], scale=1.0 / 128.0)
                gs["glx"] = glx

            def g_mlp1a():
                pm1 = psh.tile([128, W], fp32, tag="psh", name="pm1a")
                for t in range(nt):
                    sl = slice(t * f, (t + 1) * f)
                    nc.tensor.matmul(pm1[:, sl], m1_s[:, 0:128], gs["glx"][:, sl],
                                     start=True, stop=True)
                hg_t = hgp.tile([128, 2, W], fr, tag="hg", name="hg_t")
                drain(hg_t[:, 0, :], pm1, bt_s[:, 32:33])
                gs["hg"] = hg_t

            def g_mlp1b():
                pm1 = psh.tile([128, W], fp32, tag="psh", name="pm1b")
                for t in range(nt):
                    sl = slice(t * f, (t + 1) * f)
                    nc.tensor.matmul(pm1[:, sl], m1_s[:, 128:256], gs["glx"][:, sl],
                                     start=True, stop=True)
                drain(gs["hg"][:, 1, :], pm1, bt_s[:, 33:34])

            def g_mlp2():
                pzt = pz.tile([8, W], fp32, tag="pz", name="pzt")
                for t in range(nt):
                    sl = slice(t * f, (t + 1) * f)
                    for hc in range(2):
                        nc.tensor.matmul(pzt[:, sl],
                                         wgm2_s[:, hc * 8 : (hc + 1) * 8],
                                         gs["hg"][:, hc, sl],
                                         start=(hc == 0), stop=(hc == 1))
                z4 = wk.tile([8, W], fp32, tag="z4", name="z4")
                nc.vector.tensor_add(z4, pzt, gm_t)
                gs["z4"] = z4

            def g_exp():
                w4 = wk.tile([8, W], fr, tag="w4", name="w4")
                nc.scalar.activation(w4, gs["z4"], AF.Exp, bias=bt_s[0:8, 35:36])
                gs["w4"] = w4

            stages = [g_gate, g_mlp1a, g_mlp1b, g_mlp2, g_exp]

            # ---- expert branch, gating stages interleaved ----
            h2s = []
            for e in range(E):
                h1_t = hq.tile([128, 2, W], f8, tag="h1q", name="h1_t")
                for hc in range(2):
                    ph = psh.tile([128, W], fp32, tag="psh", name="ph1")
                    for t in range(nt):
                        sl = slice(t * f, (t + 1) * f)
                        nc.tensor.matmul(ph[:, sl], we1_s[:, e * 2 + hc],
                                         xs_t[:, :, sl],
                                         start=True, stop=True, perf_mode=DR)
                    drain(h1_t[:, hc, :], ph, bt_s[:, e * 2 + hc : e * 2 + hc + 1])
                h2_t = h2q.tile([128, 2, W], f8, tag="h2q", name="h2_t")
                for kc in range(2):
                    ph = psh.tile([128, W], fp32, tag="psh", name="ph2")
                    for t in range(nt):
                        sl = slice(t * f, (t + 1) * f)
                        nc.tensor.matmul(ph[:, sl], we2_s[:, e * 2 + kc],
                                         h1_t[:, :, sl],
                                         start=True, stop=True, perf_mode=DR)
                    drain(h2_t[:, kc, :],
                          ph, bt_s[:, 16 + e * 2 + kc : 17 + e * 2 + kc])
                h2s.append(h2_t)
                if e < len(stages):
                    stages[e]()
            # we3 accumulation (after the expert loop: keeps the single
            # pz-pool bank rotation acyclic: pzt -> peo -> pdn)
            peo = pz.tile([16, W], fp32, tag="pz", name="peo")
            for e in range(E):
                for t in range(nt):
                    sl = slice(t * f, (t + 1) * f)
                    nc.tensor.matmul(peo[:, sl], we3_s[:, e], h2s[e][:, :, sl],
                                     start=(e == 0), stop=(e == E - 1),
                                     perf_mode=DR)

            # ---- weighted sum rows: ewp = (peo/2048) * w ----
            ewp = wk.tile([8, W], fr, tag="ewp", name="ewp")
            nc.vector.scalar_tensor_tensor(out=ewp, in0=peo[0:8, :],
                                           scalar=1.0 / 2048.0,
                                           in1=gs["w4"].bitcast(fp32),
                                           op0=OP.mult, op1=OP.mult)
            pdn = pz.tile([2, W], fp32, tag="pz", name="pdn")
            pdn2 = psh.tile([1, W], fp32, tag="psh", name="pdn2")
            for t in range(nt):
                sl = slice(t * f, (t + 1) * f)
                nc.tensor.matmul(pdn[:, sl], dnw_s[:, 0:2],
                                 gs["w4"][:, sl], start=True, stop=True)
                nc.tensor.matmul(pdn2[:, sl], dnw_s[:, 2:3],
                                 ewp[:, sl], start=True, stop=True)
            dn_s = wk.tile([2, W], fp32, tag="dns", name="dn_s")
            nc.scalar.activation(dn_s, pdn, AF.Identity)
            nm_s = wk.tile([1, W], fp32, tag="nms", name="nm_s")
            nc.vector.tensor_copy(nm_s, pdn2)
            nc.sync.dma_start(out=scr_d.ap()[0:1, n0 : n0 + W], in_=dn_s[0:1, :])
            nc.sync.dma_start(out=scr_d.ap()[1:2, n0 : n0 + W], in_=dn_s[1:2, :])
            nc.sync.dma_start(out=scr_d.ap()[2:3, n0 : n0 + W], in_=nm_s)

        for p in range(n_pairs):
            do_pair(p, 2)
        if tail:
            do_pair(n_pairs, 1)

        # ---- post-pass: out = sigmoid((num + nb) / den), full-width ----
        dn3 = cw.tile([PP, 3, PJ], fp32)
        nc.sync.dma_start(out=dn3, in_=scr_d.ap().rearrange("c (p j) -> p c j", p=PP))
        denr = cw.tile([PP, PJ], fp32)
        nc.vector.reciprocal(denr, dn3[:, 0, :])
        nsum = cw.tile([PP, PJ], fp32)
        nc.vector.tensor_add(nsum, dn3[:, 2, :], dn3[:, 1, :])
        rat = cw.tile([PP, PJ], fp32)
        nc.vector.tensor_mul(rat, nsum, denr)
        en = cw.tile([PP, PJ], fp32)
        nc.scalar.activation(en, rat, AF.Exp, scale=-1.0)
        ep = cw.tile([PP, PJ], fp32)
        nc.vector.tensor_scalar_add(ep, en, 1.0)
        outp = cw.tile([PP, PJ], fp32)
        nc.vector.reciprocal(outp, ep)
        nc.sync.dma_start(out=out_d.ap().rearrange("(p j) -> p j", p=PP), in_=outp)

    nc.compile()
    return nc


def _q8(x):
    import ml_dtypes

    return np.clip(np.asarray(x, np.float32), -240.0, 240.0).astype(
        ml_dtypes.float8_e4m3)


def _pack_weights(ins):
    """Host-side packing of parameters into SBUF-ready layouts."""
    f32 = np.float32
    We1, be1 = np.asarray(ins["We1"], f32), np.asarray(ins["be1"], f32)
    We2, be2 = np.asarray(ins["We2"], f32), np.asarray(ins["be2"], f32)
    We3, be3 = np.asarray(ins["We3"], f32), np.asarray(ins["be3"], f32)
    Wg, bg = np.asarray(ins["Wg"], f32), np.asarray(ins["bg"], f32)
    centers = np.asarray(ins["centers"], f32)
    Wgm1, bgm1 = np.asarray(ins["Wgm1"], f32), np.asarray(ins["bgm1"], f32)
    Wgm2, bgm2 = np.asarray(ins["Wgm2"], f32), np.asarray(ins["bgm2"], f32)

    # expert weights, DoubleRow layout [p, block, kchunk, m]
    we1q = np.ascontiguousarray(
        We1.reshape(E, 2, 128, 2, 128).transpose(2, 0, 3, 1, 4)
        .reshape(128, 16 * 2 * 128))
    we2q = np.ascontiguousarray(
        (2.0 * We2).reshape(E, 2, 128, 2, 128).transpose(2, 0, 3, 1, 4)
        .reshape(128, 16 * 2 * 128))
    we3q = np.zeros((128, 8, 2, 16), f32)
    for e in range(E):
        for dc in range(2):
            we3q[:, e, dc, e] = 32.0 * We3[e, dc * 128 : (dc + 1) * 128]
    wgq = np.zeros((128, 2, 128), f32)
    wg4 = (4.0 * Wg).reshape(2, 128, L).transpose(1, 0, 2)   # [p, i, l]
    wgq[:, :, 0:64] = wg4
    wgq[:, :, 64:128] = wg4
    wgq = np.ascontiguousarray(wgq.reshape(128, 2 * 128))

    # gating mlp1 with the distance stage folded in:
    # pre1 = gl @ (2 c^T Wgm1) + glsq @ (-1 (x) colsum Wgm1) + k'
    M1 = 2.0 * centers.T @ Wgm1                      # [64, 256]
    v = Wgm1.sum(axis=0)                             # [256]
    m1ext = np.zeros((128, 256), f32)
    m1ext[0:64, :] = M1
    m1ext[64:128, :] = -np.broadcast_to(v, (64, 256))
    kp = bgm1 - (centers * centers).sum(axis=1) @ Wgm1   # [256]

    # gating mlp2, column-mean-centered for exp-range stability
    W2c = Wgm2 - Wgm2.mean(axis=1, keepdims=True)
    wgm2 = np.ascontiguousarray(
        W2c.reshape(2, 128, E).transpose(1, 0, 2).reshape(128, 2 * E))

    bt = np.zeros((128, 36), f32)
    bt[:, 0:16] = (32.0 * be1).reshape(E, 2, 128).transpose(2, 0, 1).reshape(128, 16)
    bt[:, 16:32] = (64.0 * be2).reshape(E, 2, 128).transpose(2, 0, 1).reshape(128, 16)
    bt[:, 32:34] = kp.reshape(2, 128).T
    bt[:, 34] = np.tile(bg, 2)
    bt[0:8, 35] = bgm2

    dnw = np.stack([np.ones(E, f32), be3, np.ones(E, f32)], axis=1)

    return {
        "we1": _q8(we1q), "we2": _q8(we2q),
        "we3": _q8(we3q.reshape(128, 8 * 2 * 16)), "wgq": _q8(wgq),
        "m1": m1ext, "wgm2": wgm2, "bt": bt, "dnw": dnw,
    }


def _pack_x(x, nc_n):
    """[nc_raw, 256] fp32 -> [128, 2*nc_n] fp8 (q8(32x), feature-major)."""
    nc_raw = x.shape[0]
    xq = _q8(32.0 * np.asarray(x, np.float32))
    if nc_raw < nc_n:
        pad = np.zeros((nc_n - nc_raw, 256), xq.dtype)
        xq = np.concatenate([xq, pad], axis=0)
    return np.ascontiguousarray(
        xq.T.reshape(2, 128, nc_n).transpose(1, 0, 2).reshape(128, 2 * nc_n))


def _pack_gm(gum, nc_n, f):
    """[nc_raw, 8] fp32 -> [8, nc_n]."""
    f32 = np.float32
    nc_raw = gum.shape[0]
    g = np.asarray(gum, f32)
    if nc_raw < nc_n:
        g = np.concatenate([g, np.zeros((nc_n - nc_raw, 8), f32)], axis=0)
    return np.ascontiguousarray(g.T)


def _make_in_maps(ins, nc_n=NC_N, f=F, m_cores=M_CORES):
    wmaps = _pack_weights(ins)
    f32 = np.float32
    score = np.asarray(ins["score_input"], f32)
    gate = np.asarray(ins["gate_input"], f32)
    gum = np.asarray(ins["gumbel_noise"], f32)
    nc_raw = score.shape[0] // m_cores
    in_maps = []
    for c in range(m_cores):
        s = slice(c * nc_raw, (c + 1) * nc_raw)
        m = dict(wmaps)
        m["xs"] = _pack_x(score[s], nc_n)
        m["xg"] = _pack_x(gate[s], nc_n)
        m["gmp"] = _pack_gm(gum[s], nc_n, f)
        in_maps.append(m)
    return in_maps


_NC_CACHE = {}


def _get_nc(nc_n, f):
    key = (nc_n, f)
    if key not in _NC_CACHE:
        _NC_CACHE[key] = _build_nc(nc_n, f)
    return _NC_CACHE[key]


def kernel(**inputs) -> np.ndarray:
    from concourse.bass_utils import run_bass_kernel_spmd

    nc = _get_nc(NC_N, F)
    in_maps = _make_in_maps(inputs)
    res = run_bass_kernel_spmd(nc, in_maps, core_ids=list(range(M_CORES)))
    out = np.concatenate(
        [res.results[c]["out"][:NC_RAW] for c in range(M_CORES)])
    return out.reshape(N, 1).astype(np.float32)


if __name__ == "__main__":
    import jax

    with jax.default_device(jax.local_devices(backend="cpu")[0]):
        import reference

        ins = reference.setup_inputs()
        ins = {k: np.asarray(v) for k, v in ins.items()}
        expected = np.asarray(reference.reference(**ins))
    out = kernel(**ins)
    err = np.abs(out - expected).max()
    print("max abs err:", err, "rel:", err / np.abs(expected).max())
